# revision 7
# baseline (speedup 1.0000x reference)
"""DynamicConv (MoE-routed 1x1 conv) Trainium2 kernel.

Data-parallel over batch: 8 cores x 4 samples each. All HBM streams are
f16 (host casts x/weights down and the f16 output back up), which halves
DMA traffic vs f32 and keeps the kernel near the compute/memory ridge.
Per core:
  - routing MLP (3-layer, exact GELU) in transposed orientation; softmax
    numerator/denominator are split: unnormalized exps scale the kernel
    mix and 1/sum is folded into the PSUM eviction (scale+bias, one op).
  - exp is computed as (1+tanh(x/2))/(1-tanh(x/2)); tanh shares the act
    table with exact Gelu so only one table load exists in the kernel.
  - logits reach all 128 partitions with no SBUF->SBUF DMA: a W2*h2
    broadcast-AP product, a ones-matmul collapse, and a gpsimd
    partition_broadcast.
  - expert kernels are mixed per sample with f32 AXPY chains on DVE
    (GPSIMD cannot run TensorScalarPtr on real TRN2), f16 results.
  - main matmuls on PE in f16 (1 cycle/row) into f32 PSUM, 4 rotating
    2-bank PSUM chunks; evictions split ACT-heavy/DVE-late; out-writes
    spread over Pool/SP so the last bytes leave on idle queues.

Cost-model HW time: 42.1us (baseline 67.7us). Measured rel err 4.1e-4.

Hardcoded problem: x[32,256,4096] f32, embedding[32,128] f32,
W0[128,128] b0[128] W1[128,128] b1[128] W2[128,8] b2[8],
weight[8,256,256,1], bias_k[8,256] -> out[32,256,4096] f32.
"""

import numpy as np

import concourse.bacc as bacc
import concourse.mybir as mybir
import concourse.tile as tile
from concourse import bass_utils

F32 = mybir.dt.float32
F16 = mybir.dt.float16
AF = mybir.ActivationFunctionType
ALU = mybir.AluOpType

N_CORES = 8
BS = 32
BPC = BS // N_CORES
IN_C = 256
OUT_C = 256
H = 4096
K = 8
D_EMBD = 128
HID = 128
N_IT = IN_C // 128
N_OT = OUT_C // 128

# params tile 1a/1b (arrive first; routing MLP front). exp(b2) is folded
# into wta and bias_k on the host, so the row path broadcasts raw exp(l)
# and the softmax denominator comes from an expb2-weighted 8x4 matmul.
P1_W0 = 0                 # [128, 128] W0[d, h]
P1_EMBT = 128             # [128, 4]  emb.T
P1_B0 = 132               # [128, 1]
P1A_COLS = 133
P1B_W1 = 0                # [128, 128] W1
P1B_B1 = 128              # [128, 1]
P1B_ONES = 129            # [128, 1] all ones
P1B_COLS = 130

# params tile 2
P2_W2 = 0                 # [128, 8] W2[h, k]
P2_EB2C = 8               # rows 0:8, col: exp(b2[k])
P2_BK = 9                 # rows 0:8, [8, 256] exp(b2[k])*bias_k
P2_COLS = 265

# evict engine by global chunk order index g: DVE (the dedicated evictor)
# takes odd g plus a few evens; ACT the rest. The final chunk (g=31) is
# split into two 512-col halves done by ACT and DVE concurrently.
# explicit evictor assignment by global chunk order index (g31 is split)
DVE_EV = {17, 19, 21, 23, 25, 27, 29}


def ev_on_dve(g):
    return g in DVE_EV

_PROG = None


def _build_program():
    nc = bacc.Bacc("TRN2", target_bir_lowering=False, debug=False)

    xs = nc.dram_tensor("xs", [BPC, IN_C, H], F16, kind="ExternalInput").ap()
    # wta[il, it*2048 + k*256 + o] = weight[k, o, it*128+il]
    wta = nc.dram_tensor("wta", [128, N_IT * K * OUT_C], F16,
                         kind="ExternalInput").ap()
    params1a = nc.dram_tensor("params1a", [128, P1A_COLS], F32,
                              kind="ExternalInput").ap()
    params1b = nc.dram_tensor("params1b", [128, P1B_COLS], F32,
                              kind="ExternalInput").ap()
    params2 = nc.dram_tensor("params2", [128, P2_COLS], F32,
                             kind="ExternalInput").ap()
    out = nc.dram_tensor("out", [BPC, OUT_C, H], F16, kind="ExternalOutput").ap()

    with tile.TileContext(nc) as tc:
        with (
            tc.tile_pool(name="consts", bufs=1) as cpool,
            tc.tile_pool(name="mix32", bufs=1) as mx32,
            tc.tile_pool(name="mixt", bufs=2) as mxt,
            tc.tile_pool(name="mix16", bufs=4) as mx16,
            tc.tile_pool(name="xin", bufs=4) as xinp,
            tc.tile_pool(name="osb", bufs=4) as osbp,
            tc.tile_pool(name="o3sb", bufs=4) as o3sbp,
            tc.tile_pool(name="mps", bufs=1, space="PSUM") as mps,
        ):
            # ---- param + weight loads: pa1 + it0-wta on Pool, pa2 +
            # it1-wta on SP (then SP streams x). Keeps every queue's
            # first-needed bytes at its front.
            pa1 = cpool.tile([128, P1A_COLS], F32, tag="pa1")
            nc.gpsimd.dma_start(pa1[:], params1a[:])
            pa1b = cpool.tile([128, P1B_COLS], F32, tag="pa1b")
            nc.gpsimd.dma_start(pa1b[:], params1b[:])
            pa2 = cpool.tile([128, P2_COLS], F32, tag="pa2")
            nc.gpsimd.dma_start(pa2[:], params2[:])

            wt_q = []
            for q in range(4):
                t = cpool.tile([128, 1024], F16, tag=f"wtq{q}", name=f"wtq{q}")
                eng = nc.sync if q < 2 else nc.gpsimd
                eng.dma_start(t[:], wta[:, q * 1024:(q + 1) * 1024])
                wt_q.append(t)

            # ---- ACT Gelu table warmup (Exp loads once, before eT/erow;
            # keep every Gelu use before the first Exp use) ----
            warm = cpool.tile([128, 1], F32, tag="warm")
            nc.vector.memset(warm[:], 0.25)
            warm_o = cpool.tile([128, 1], F32, tag="warm_o")
            nc.scalar.activation(warm_o[:], warm[:], AF.Gelu)

            def wt_sb(k, it):
                q = it * 2 + k // 4
                off = (k % 4) * 256
                return wt_q[q][:, off:off + 256]

            # ---- routing MLP (transposed; all samples at once) ----
            # routing PSUMs live in slices of one pm0 tile; the WAW chain
            # through it serializes them naturally and frees no extra bank
            rpt = mps.tile([128, 1024], F32, tag="pm0", name="rpt")
            p1 = rpt[:, 0:BPC]
            nc.tensor.matmul(p1, pa1[:, P1_W0:P1_W0 + HID],
                             pa1[:, P1_EMBT:P1_EMBT + BPC], start=True, stop=True)
            h1 = cpool.tile([HID, BPC], F32, tag="h1")
            nc.scalar.activation(h1[:], p1, AF.Gelu,
                                 bias=pa1[:, P1_B0:P1_B0 + 1])

            p2 = rpt[:, 8:8 + BPC]
            nc.tensor.matmul(p2, pa1b[:, P1B_W1:P1B_W1 + HID], h1[:],
                             start=True, stop=True)
            h2 = cpool.tile([HID, BPC], F32, tag="h2")
            nc.scalar.activation(h2[:], p2, AF.Gelu,
                                 bias=pa1b[:, P1B_B1:P1B_B1 + 1])

            # column path: eT[k, b] = exp(l[k, b] + b2[k])  (for agg bias)
            p3 = rpt[0:K, 16:16 + BPC]
            nc.tensor.matmul(p3, pa2[:, P2_W2:P2_W2 + K], h2[:],
                             start=True, stop=True)
            # exp(x) = (1 + tanh(x/2)) / (1 - tanh(x/2)): tanh lives in the
            # same act table as exact Gelu, so no second table load.
            eT = cpool.tile([K, BPC], F32, tag="eT")
            tcol = cpool.tile([K, BPC], F32, tag="tcol")
            nc.scalar.activation(tcol[:], p3, AF.Tanh, scale=0.5)
            dencol = cpool.tile([K, BPC], F32, tag="dencol")
            nc.vector.tensor_scalar(dencol[:], tcol[:], -1.0, 1.0,
                                    op0=ALU.mult, op1=ALU.add)
            nc.vector.reciprocal(dencol[:], dencol[:])
            numcol = cpool.tile([K, BPC], F32, tag="numcol")
            nc.vector.tensor_scalar(numcol[:], tcol[:], 1.0, None, op0=ALU.add)
            nc.vector.tensor_tensor(eT[:], numcol[:], dencol[:], op=ALU.mult)

            # row path: l_row[0, (b,k)] = sum_h W2[h,k] * h2[h,b]
            prod = cpool.tile([128, BPC * K], F32, tag="prod")
            w2b = pa2[:, P2_W2:P2_W2 + K].unsqueeze(1).broadcast_to([128, BPC, K])
            h2b = h2[:].unsqueeze(2).broadcast_to([128, BPC, K])
            nc.vector.tensor_tensor(
                prod[:].rearrange("p (b k) -> p b k", b=BPC), w2b, h2b,
                op=ALU.mult)
            lrow = rpt[0:1, 32:32 + BPC * K]
            nc.tensor.matmul(lrow, pa1b[:, P1B_ONES:P1B_ONES + 1], prod[:],
                             start=True, stop=True)
            erow = cpool.tile([1, BPC * K], F32, tag="erow")
            trow = cpool.tile([1, BPC * K], F32, tag="trow")
            nc.scalar.activation(trow[:], lrow, AF.Tanh, scale=0.5)
            denrow = cpool.tile([1, BPC * K], F32, tag="denrow")
            nc.vector.tensor_scalar(denrow[:], trow[:], -1.0, 1.0,
                                    op0=ALU.mult, op1=ALU.add)
            nc.vector.reciprocal(denrow[:], denrow[:])
            numrow = cpool.tile([1, BPC * K], F32, tag="numrow")
            nc.vector.tensor_scalar(numrow[:], trow[:], 1.0, None, op0=ALU.add)
            nc.vector.tensor_tensor(erow[:], numrow[:], denrow[:], op=ALU.mult)

            # broadcast raw exps immediately: the mix needs only these
            eB = cpool.tile([128, BPC * K], F32, tag="eB")
            nc.gpsimd.partition_broadcast(eB[:], erow[:])

            # softmax denominator: s[1, b] = sum_k exp(b2[k]) * eT[k, b]
            srow = rpt[0:1, 64:64 + BPC]
            nc.tensor.matmul(srow, pa2[0:K, P2_EB2C:P2_EB2C + 1], eT[:],
                             start=True, stop=True)
            rrow = cpool.tile([1, BPC], F32, tag="rrow")
            nc.vector.reciprocal(rrow[:], srow)
            rBt = cpool.tile([128, BPC], F32, tag="rBt")
            nc.gpsimd.partition_broadcast(rBt[:], rrow[:])

            def sc(b, k):
                return eB[:, b * K + k:b * K + k + 1]

            # agg bias column-path matmuls
            pag0 = rpt[:, 128:128 + BPC]
            nc.tensor.matmul(pag0, pa2[0:K, P2_BK:P2_BK + 128], eT[:],
                             start=True, stop=True)
            pag1 = rpt[:, 256:256 + BPC]
            nc.tensor.matmul(pag1, pa2[0:K, P2_BK + 128:P2_BK + 256], eT[:],
                             start=True, stop=True)

            aggb = cpool.tile([128, N_OT * BPC], F32, tag="aggb")

            # ---- mix chains ----
            # pre-allocate result tiles in sample order (bufs=4: no recycling)
            mh = {}
            for b in range(BPC):
                for it in range(N_IT):
                    mh[(b, it)] = mx16.tile([128, 256], F16, tag=f"m16_{it}",
                                            name=f"m16_{b}_{it}")

            def mix_chain(eng, b, it, lo, hi):
                m32 = mx32.tile([128, 256], F32, tag=f"m32_{it}",
                                name=f"m32_{b}_{it}_{lo}")
                m16 = mh[(b, it)]
                eng.tensor_scalar_mul(m32[:, lo:hi], wt_sb(0, it)[:, lo:hi],
                                      sc(b, 0))
                for k in range(1, K - 1):
                    eng.scalar_tensor_tensor(
                        m32[:, lo:hi], wt_sb(k, it)[:, lo:hi], sc(b, k),
                        m32[:, lo:hi], op0=ALU.mult, op1=ALU.add)
                eng.scalar_tensor_tensor(
                    m16[:, lo:hi], wt_sb(K - 1, it)[:, lo:hi], sc(b, K - 1),
                    m32[:, lo:hi], op0=ALU.mult, op1=ALU.add)

            def mix_tree(eng, b, it):
                """f16 4x-mode muls + 2x-mode pairwise adds (wide chains)."""
                m16 = mh[(b, it)]
                ts = []
                for k in range(K):
                    t = mxt.tile([128, 256], F16, tag=f"mt{k}",
                                 name=f"mt_{b}_{it}_{k}")
                    eng.tensor_scalar_mul(t[:], wt_sb(k, it), sc(b, k))
                    ts.append(t)
                eng.tensor_tensor(ts[0][:], ts[0][:], ts[1][:], op=ALU.add)
                eng.tensor_tensor(ts[2][:], ts[2][:], ts[3][:], op=ALU.add)
                eng.tensor_tensor(ts[4][:], ts[4][:], ts[5][:], op=ALU.add)
                eng.tensor_tensor(ts[6][:], ts[6][:], ts[7][:], op=ALU.add)
                eng.tensor_tensor(ts[0][:], ts[0][:], ts[2][:], op=ALU.add)
                eng.tensor_tensor(ts[4][:], ts[4][:], ts[6][:], op=ALU.add)
                eng.tensor_tensor(m16[:], ts[0][:], ts[4][:], op=ALU.add)

            # aggb normalization first: evicts depend on it, and the DVE
            # list-scheduler won't hoist it past the chains below
            nc.vector.tensor_tensor(aggb[:, 0:BPC], pag0, rBt[:],
                                    op=ALU.mult)
            nc.vector.tensor_tensor(aggb[:, BPC:2 * BPC], pag1, rBt[:],
                                    op=ALU.mult)

            # All mix chains on DVE (GPSIMD cannot run TensorScalarPtr on
            # real TRN2); b0's chains are ot-split for an earlier PE start.
            mix_chain(nc.vector, 0, 0, 0, 128)
            mix_chain(nc.vector, 0, 1, 0, 128)
            mix_chain(nc.vector, 0, 0, 128, 256)
            mix_chain(nc.vector, 0, 1, 128, 256)
            mix_chain(nc.vector, 1, 0, 0, 256)
            mix_chain(nc.vector, 1, 1, 0, 256)
            mix_chain(nc.vector, 2, 0, 0, 256)
            mix_chain(nc.vector, 2, 1, 0, 256)
            mix_chain(nc.vector, 3, 0, 0, 256)
            mix_chain(nc.vector, 3, 1, 0, 256)

            # ---- x loads (SP): per sample, per it, two 2048-col halves ----
            x_t = {}
            for b in range(BPC):
                for half in range(2):
                    for it in range(N_IT):
                        t = xinp.tile([128, 2048], F16, tag=f"x{it}{half}",
                                      name=f"x_{b}_{it}_{half}")
                        x_t[(b, it, half)] = t
            for b in range(BPC):
                if b == 0:
                    # interleaved 1024-col quarters: x arrives in the order
                    # the first sample's psum chunks consume it
                    for qq in range(4):
                        for it in range(N_IT):
                            t = x_t[(b, it, qq // 2)]
                            lo = (qq % 2) * 1024
                            nc.sync.dma_start(
                                t[:, lo:lo + 1024],
                                xs[b, it * 128:(it + 1) * 128,
                                   qq * 1024:(qq + 1) * 1024])
                else:
                    for half in range(2):
                        for it in range(N_IT):
                            t = x_t[(b, it, half)]
                            nc.sync.dma_start(
                                t[:], xs[b, it * 128:(it + 1) * 128,
                                         half * 2048:(half + 1) * 2048])

            osb = {}
            for b in range(BPC):
                for ot in range(N_OT):
                    if (b, ot) == (3, 1):
                        for q in range(3):
                            osb[(b, ot, q)] = o3sbp.tile(
                                [128, 1024], F16, tag="o3",
                                name=f"o3_{b}_{ot}_{q}")
                        for sq in range(2):
                            osb[(b, ot, 3, sq)] = o3sbp.tile(
                                [128, 512], F16, tag="o3s",
                                name=f"o3s_{b}_{ot}_{sq}")
                    else:
                        for hh in range(2):
                            osb[(b, ot, hh)] = osbp.tile(
                                [128, 2048], F16, tag="o",
                                name=f"o_{b}_{ot}_{hh}")

            def emit_chunk(g, b, ot, c):
                """Matmuls + eviction for one 1024-col psum chunk."""
                ps = mps.tile([128, 1024], F32, tag=f"pm{g % 4}",
                              name=f"ps_{b}_{ot}_{c}")
                half = c // 2
                for s in range(2):
                    lo = (c % 2) * 1024 + s * 512
                    for it in range(N_IT):
                        nc.tensor.matmul(
                            ps[:, s * 512:(s + 1) * 512],
                            mh[(b, it)][:, ot * 128:(ot + 1) * 128],
                            x_t[(b, it, half)][:, lo:lo + 512],
                            start=(it == 0), stop=(it == N_IT - 1))
                bias_ap = aggb[:, ot * BPC + b:ot * BPC + b + 1]
                scale_ap = rBt[:, b:b + 1]
                if g == 31:
                    # final chunk: two 512-col halves, separate out tiles,
                    # evicted concurrently on ACT and DVE
                    nc.scalar.activation(osb[(b, ot, 3, 0)][:],
                                         ps[:, 0:512], AF.Identity,
                                         bias=bias_ap, scale=scale_ap)
                    nc.vector.tensor_scalar(osb[(b, ot, 3, 1)][:],
                                            ps[:, 512:1024],
                                            scale_ap, bias_ap,
                                            op0=ALU.mult, op1=ALU.add)
                    return
                if (b, ot) == (3, 1):
                    otile, olo = osb[(b, ot, c)], 0
                else:
                    otile, olo = osb[(b, ot, c // 2)], (c % 2) * 1024
                dst = otile[:, olo:olo + 1024]
                if ev_on_dve(g):
                    nc.vector.tensor_scalar(dst, ps[:], scale_ap, bias_ap,
                                            op0=ALU.mult, op1=ALU.add)
                else:
                    nc.scalar.activation(dst, ps[:], AF.Identity,
                                         bias=bias_ap, scale=scale_ap)

            def emit_writes(b, ot):
                # ACT: early halves (slack between its evicts); Pool: mid
                # halves deferred past its mix chains; SP: all of b3 (its x
                # stream is done by then).
                if (b, ot) == (3, 1):
                    for q in range(3):
                        orows = out[b, ot * 128:(ot + 1) * 128,
                                    q * 1024:(q + 1) * 1024]
                        nc.sync.dma_start(orows, osb[(b, ot, q)][:])
                    for sq in range(2):
                        orows = out[b, ot * 128:(ot + 1) * 128,
                                    3072 + sq * 512:3072 + (sq + 1) * 512]
                        nc.sync.dma_start(orows, osb[(b, ot, 3, sq)][:])
                    return
                for hh in range(2):
                    orows = out[b, ot * 128:(ot + 1) * 128,
                                hh * 2048:(hh + 1) * 2048]
                    src = osb[(b, ot, hh)][:]
                    nc.gpsimd.dma_start(orows, src)

            # chunk order per sample: h0 chunks of both ots first, then h1
            # (x's second half is needed 3.4us into the sample, giving the
            # serial SP x-stream slack to stay ahead of PE)
            g = 0
            B0_ORDER = ((0, 0), (1, 0), (0, 1), (1, 1),
                        (0, 2), (1, 2), (0, 3), (1, 3))
            ORDER = ((0, 0), (0, 1), (1, 0), (1, 1),
                     (0, 2), (0, 3), (1, 2), (1, 3))
            for b in range(BPC):
                for ot, c in (B0_ORDER if b == 0 else ORDER):
                    emit_chunk(g, b, ot, c)
                    g += 1
                emit_writes(b, 0)
                emit_writes(b, 1)

    nc.compile()
    return nc


def _get_program():
    global _PROG
    if _PROG is None:
        _PROG = _build_program()
    return _PROG


def build_in_maps(inputs):
    x = np.asarray(inputs["x"], dtype=np.float32)
    emb = np.asarray(inputs["embedding"], dtype=np.float32)
    W0 = np.asarray(inputs["W0"], dtype=np.float32)
    b0 = np.asarray(inputs["b0"], dtype=np.float32)
    W1 = np.asarray(inputs["W1"], dtype=np.float32)
    b1 = np.asarray(inputs["b1"], dtype=np.float32)
    W2 = np.asarray(inputs["W2"], dtype=np.float32)
    b2 = np.asarray(inputs["b2"], dtype=np.float32)
    weight = np.asarray(inputs["weight"], dtype=np.float32)[..., 0]  # [K,O,I]
    bias_k = np.asarray(inputs["bias_k"], dtype=np.float32)

    x16 = x.astype(np.float16)

    expb2 = np.exp(b2.astype(np.float64))
    wscaled = weight * expb2[:, None, None].astype(np.float32)
    wta = np.ascontiguousarray(
        wscaled.transpose(2, 0, 1)            # [I, K, O]
        .reshape(N_IT, 128, K, OUT_C)         # [it, il, K, O]
        .transpose(1, 0, 2, 3)                # [il, it, K, O]
        .reshape(128, N_IT * K * OUT_C)).astype(np.float16)

    pa1 = np.zeros((128, P1A_COLS), dtype=np.float32)
    pa1[:, P1_W0:P1_W0 + HID] = W0
    pa1[:, P1_B0] = b0
    pa1b = np.zeros((128, P1B_COLS), dtype=np.float32)
    pa1b[:, P1B_W1:P1B_W1 + HID] = W1
    pa1b[:, P1B_B1] = b1
    pa1b[:, P1B_ONES] = 1.0

    pa2 = np.zeros((128, P2_COLS), dtype=np.float32)
    pa2[:, P2_W2:P2_W2 + K] = W2
    pa2[0:K, P2_EB2C] = expb2.astype(np.float32)
    pa2[0:K, P2_BK:P2_BK + OUT_C] = (
        bias_k * expb2[:, None]).astype(np.float32)

    in_maps = []
    for c in range(N_CORES):
        sl = slice(c * BPC, (c + 1) * BPC)
        p1 = pa1.copy()
        p1[:, P1_EMBT:P1_EMBT + BPC] = emb[sl].T
        in_maps.append({
            "xs": np.ascontiguousarray(x16[sl]),
            "wta": wta,
            "params1a": p1,
            "params1b": pa1b,
            "params2": pa2,
        })
    return in_maps


def run(inputs, trace=False):
    nc = _get_program()
    in_maps = build_in_maps(inputs)
    res = bass_utils.run_bass_kernel_spmd(
        nc, in_maps, core_ids=list(range(N_CORES)), trace=trace)
    out = np.concatenate(
        [res.results[c]["out"] for c in range(N_CORES)], axis=0
    ).astype(np.float32)
    return out, res


def kernel(**inputs):
    out, _ = run(inputs, trace=False)
    return out


# revision 8
# speedup vs baseline: 1.0071x; 1.0071x over previous
"""DynamicConv (MoE-routed 1x1 conv) Trainium2 kernel.

Data-parallel over batch: 8 cores x 4 samples each. All HBM streams are
f16 (host casts x/weights down and the f16 output back up), which halves
DMA traffic vs f32 and keeps the kernel near the compute/memory ridge.
Per core:
  - routing MLP (3-layer, exact GELU) in transposed orientation; softmax
    numerator/denominator are split: unnormalized exps scale the kernel
    mix and 1/sum is folded into the PSUM eviction (scale+bias, one op).
  - exp is computed as (1+tanh(x/2))/(1-tanh(x/2)); tanh shares the act
    table with exact Gelu so only one table load exists in the kernel.
  - logits reach all 128 partitions with no SBUF->SBUF DMA: a W2*h2
    broadcast-AP product, a ones-matmul collapse, and a gpsimd
    partition_broadcast.
  - expert kernels are mixed per sample with f32 AXPY chains on DVE
    (GPSIMD cannot run TensorScalarPtr on real TRN2), f16 results.
  - main matmuls on PE in f16 (1 cycle/row) into f32 PSUM, 4 rotating
    2-bank PSUM chunks; evictions split ACT-heavy/DVE-late; out-writes
    spread over Pool/SP so the last bytes leave on idle queues.

Cost-model HW time: 42.1us (baseline 67.7us). Measured rel err 4.1e-4.

Hardcoded problem: x[32,256,4096] f32, embedding[32,128] f32,
W0[128,128] b0[128] W1[128,128] b1[128] W2[128,8] b2[8],
weight[8,256,256,1], bias_k[8,256] -> out[32,256,4096] f32.
"""

import numpy as np

import concourse.bacc as bacc
import concourse.mybir as mybir
import concourse.tile as tile
from concourse import bass_utils

F32 = mybir.dt.float32
F16 = mybir.dt.float16
AF = mybir.ActivationFunctionType
ALU = mybir.AluOpType

N_CORES = 8
BS = 32
BPC = BS // N_CORES
IN_C = 256
OUT_C = 256
H = 4096
K = 8
D_EMBD = 128
HID = 128
N_IT = IN_C // 128
N_OT = OUT_C // 128

# params tile 1a/1b (arrive first; routing MLP front). exp(b2) is folded
# into wta and bias_k on the host, so the row path broadcasts raw exp(l)
# and the softmax denominator comes from an expb2-weighted 8x4 matmul.
P1_W0 = 0                 # [128, 128] W0[d, h]
P1_EMBT = 128             # [128, 4]  emb.T
P1_B0 = 132               # [128, 1]
P1A_COLS = 133
P1B_W1 = 0                # [128, 128] W1
P1B_B1 = 128              # [128, 1]
P1B_ONES = 129            # [128, 1] all ones
P1B_COLS = 130

# params tile 2
P2_W2 = 0                 # [128, 8] W2[h, k]
P2_EB2C = 8               # rows 0:8, col: exp(b2[k])
P2_BK = 9                 # rows 0:8, [8, 256] exp(b2[k])*bias_k
P2_MSK = 265              # rows 0:8, [8, 32] tiled I8 selector
P2_COLS = 297

# evict engine by global chunk order index g: DVE (the dedicated evictor)
# takes odd g plus a few evens; ACT the rest. The final chunk (g=31) is
# split into two 512-col halves done by ACT and DVE concurrently.
# explicit evictor assignment by global chunk order index (g31 is split)
DVE_EV = {17, 19, 21, 23, 25, 27, 29}


def ev_on_dve(g):
    return g in DVE_EV

_PROG = None


def _build_program():
    nc = bacc.Bacc("TRN2", target_bir_lowering=False, debug=False)

    xs = nc.dram_tensor("xs", [BPC, IN_C, H], F16, kind="ExternalInput").ap()
    # wta[il, it*2048 + k*256 + o] = weight[k, o, it*128+il]
    wta = nc.dram_tensor("wta", [128, N_IT * K * OUT_C], F16,
                         kind="ExternalInput").ap()
    params1a = nc.dram_tensor("params1a", [128, P1A_COLS], F32,
                              kind="ExternalInput").ap()
    params1b = nc.dram_tensor("params1b", [128, P1B_COLS], F32,
                              kind="ExternalInput").ap()
    params2 = nc.dram_tensor("params2", [128, P2_COLS], F32,
                             kind="ExternalInput").ap()
    out = nc.dram_tensor("out", [BPC, OUT_C, H], F16, kind="ExternalOutput").ap()

    with tile.TileContext(nc) as tc:
        with (
            tc.tile_pool(name="consts", bufs=1) as cpool,
            tc.tile_pool(name="mix32", bufs=1) as mx32,
            tc.tile_pool(name="mixt", bufs=2) as mxt,
            tc.tile_pool(name="mix16", bufs=4) as mx16,
            tc.tile_pool(name="xin", bufs=4) as xinp,
            tc.tile_pool(name="osb", bufs=4) as osbp,
            tc.tile_pool(name="o3sb", bufs=4) as o3sbp,
            tc.tile_pool(name="mps", bufs=1, space="PSUM") as mps,
        ):
            # ---- param + weight loads: pa1 + it0-wta on Pool, pa2 +
            # it1-wta on SP (then SP streams x). Keeps every queue's
            # first-needed bytes at its front.
            pa1 = cpool.tile([128, P1A_COLS], F32, tag="pa1")
            nc.gpsimd.dma_start(pa1[:], params1a[:])
            pa1b = cpool.tile([128, P1B_COLS], F32, tag="pa1b")
            nc.gpsimd.dma_start(pa1b[:], params1b[:])
            pa2 = cpool.tile([128, P2_COLS], F32, tag="pa2")
            nc.gpsimd.dma_start(pa2[:], params2[:])

            wt_q = []
            for q in range(4):
                t = cpool.tile([128, 1024], F16, tag=f"wtq{q}", name=f"wtq{q}")
                eng = nc.sync if q < 2 else nc.gpsimd
                eng.dma_start(t[:], wta[:, q * 1024:(q + 1) * 1024])
                wt_q.append(t)

            # ---- ACT Gelu table warmup (Exp loads once, before eT/erow;
            # keep every Gelu use before the first Exp use) ----
            warm = cpool.tile([128, 1], F32, tag="warm")
            nc.vector.memset(warm[:], 0.25)
            warm_o = cpool.tile([128, 1], F32, tag="warm_o")
            nc.scalar.activation(warm_o[:], warm[:], AF.Gelu)

            def wt_sb(k, it):
                q = it * 2 + k // 4
                off = (k % 4) * 256
                return wt_q[q][:, off:off + 256]

            # ---- routing MLP (transposed; all samples at once) ----
            # routing PSUMs live in slices of one pm0 tile; the WAW chain
            # through it serializes them naturally and frees no extra bank
            rpt = mps.tile([128, 1024], F32, tag="pm0", name="rpt")
            p1 = rpt[:, 0:BPC]
            nc.tensor.matmul(p1, pa1[:, P1_W0:P1_W0 + HID],
                             pa1[:, P1_EMBT:P1_EMBT + BPC], start=True, stop=True)
            h1 = cpool.tile([HID, BPC], F32, tag="h1")
            nc.scalar.activation(h1[:], p1, AF.Gelu,
                                 bias=pa1[:, P1_B0:P1_B0 + 1])

            p2 = rpt[:, 8:8 + BPC]
            nc.tensor.matmul(p2, pa1b[:, P1B_W1:P1B_W1 + HID], h1[:],
                             start=True, stop=True)
            h2 = cpool.tile([HID, BPC], F32, tag="h2")
            nc.scalar.activation(h2[:], p2, AF.Gelu,
                                 bias=pa1b[:, P1B_B1:P1B_B1 + 1])

            # column path: eT[k, b] = exp(l[k, b] + b2[k])  (for agg bias)
            p3 = rpt[0:K, 16:16 + BPC]
            nc.tensor.matmul(p3, pa2[:, P2_W2:P2_W2 + K], h2[:],
                             start=True, stop=True)
            # exp(x) = (1 + tanh(x/2)) / (1 - tanh(x/2)): tanh lives in the
            # same act table as exact Gelu, so no second table load.
            eT = cpool.tile([K, BPC], F32, tag="eT")
            tcol = cpool.tile([K, BPC], F32, tag="tcol")
            nc.scalar.activation(tcol[:], p3, AF.Tanh, scale=0.5)
            dencol = cpool.tile([K, BPC], F32, tag="dencol")
            nc.vector.tensor_scalar(dencol[:], tcol[:], -1.0, 1.0,
                                    op0=ALU.mult, op1=ALU.add)
            nc.vector.reciprocal(dencol[:], dencol[:])
            numcol = cpool.tile([K, BPC], F32, tag="numcol")
            nc.vector.tensor_scalar(numcol[:], tcol[:], 1.0, None, op0=ALU.add)
            nc.vector.tensor_tensor(eT[:], numcol[:], dencol[:], op=ALU.mult)

            # row path: l_row[0, (b,k)] = sum_h W2[h,k] * h2[h,b]
            prod = cpool.tile([128, BPC * K], F32, tag="prod")
            w2b = pa2[:, P2_W2:P2_W2 + K].unsqueeze(1).broadcast_to([128, BPC, K])
            h2b = h2[:].unsqueeze(2).broadcast_to([128, BPC, K])
            nc.vector.tensor_tensor(
                prod[:].rearrange("p (b k) -> p b k", b=BPC), w2b, h2b,
                op=ALU.mult)
            lrow = rpt[0:1, 32:32 + BPC * K]
            nc.tensor.matmul(lrow, pa1b[:, P1B_ONES:P1B_ONES + 1], prod[:],
                             start=True, stop=True)
            erow = cpool.tile([1, BPC * K], F32, tag="erow")
            trow = cpool.tile([1, BPC * K], F32, tag="trow")
            nc.scalar.activation(trow[:], lrow, AF.Tanh, scale=0.5)
            denrow = cpool.tile([1, BPC * K], F32, tag="denrow")
            nc.vector.tensor_scalar(denrow[:], trow[:], -1.0, 1.0,
                                    op0=ALU.mult, op1=ALU.add)
            nc.vector.reciprocal(denrow[:], denrow[:])
            numrow = cpool.tile([1, BPC * K], F32, tag="numrow")
            nc.vector.tensor_scalar(numrow[:], trow[:], 1.0, None, op0=ALU.add)
            nc.vector.tensor_tensor(erow[:], numrow[:], denrow[:], op=ALU.mult)

            # broadcast raw exps immediately: the mix needs only these
            eB = cpool.tile([128, BPC * K], F32, tag="eB")
            nc.gpsimd.partition_broadcast(eB[:], erow[:])

            # softmax denominator: s[1, b] = sum_k exp(b2[k]) * eT[k, b]
            srow = rpt[0:1, 64:64 + BPC]
            nc.tensor.matmul(srow, pa2[0:K, P2_EB2C:P2_EB2C + 1], eT[:],
                             start=True, stop=True)
            rrow = cpool.tile([1, BPC], F32, tag="rrow")
            nc.vector.reciprocal(rrow[:], srow)
            rBt = cpool.tile([128, BPC], F32, tag="rBt")
            nc.gpsimd.partition_broadcast(rBt[:], rrow[:])

            def sc(b, k):
                return eB[:, b * K + k:b * K + k + 1]

            # agg bias column-path matmuls
            pag0 = rpt[:, 128:128 + BPC]
            nc.tensor.matmul(pag0, pa2[0:K, P2_BK:P2_BK + 128], eT[:],
                             start=True, stop=True)
            pag1 = rpt[:, 256:256 + BPC]
            nc.tensor.matmul(pag1, pa2[0:K, P2_BK + 128:P2_BK + 256], eT[:],
                             start=True, stop=True)

            aggb = cpool.tile([128, N_OT * BPC], F32, tag="aggb")

            # ---- mix chains ----
            # pre-allocate result tiles in sample order (bufs=4: no recycling)
            mh = {}
            for b in range(BPC):
                for it in range(N_IT):
                    mh[(b, it)] = mx16.tile([128, 256], F16, tag=f"m16_{it}",
                                            name=f"m16_{b}_{it}")

            def mix_chain(eng, b, it, lo, hi):
                m32 = mx32.tile([128, 256], F32, tag=f"m32_{it}",
                                name=f"m32_{b}_{it}_{lo}")
                m16 = mh[(b, it)]
                eng.tensor_scalar_mul(m32[:, lo:hi], wt_sb(0, it)[:, lo:hi],
                                      sc(b, 0))
                for k in range(1, K - 1):
                    eng.scalar_tensor_tensor(
                        m32[:, lo:hi], wt_sb(k, it)[:, lo:hi], sc(b, k),
                        m32[:, lo:hi], op0=ALU.mult, op1=ALU.add)
                eng.scalar_tensor_tensor(
                    m16[:, lo:hi], wt_sb(K - 1, it)[:, lo:hi], sc(b, K - 1),
                    m32[:, lo:hi], op0=ALU.mult, op1=ALU.add)

            def mix_tree(eng, b, it):
                """f16 4x-mode muls + 2x-mode pairwise adds (wide chains)."""
                m16 = mh[(b, it)]
                ts = []
                for k in range(K):
                    t = mxt.tile([128, 256], F16, tag=f"mt{k}",
                                 name=f"mt_{b}_{it}_{k}")
                    eng.tensor_scalar_mul(t[:], wt_sb(k, it), sc(b, k))
                    ts.append(t)
                eng.tensor_tensor(ts[0][:], ts[0][:], ts[1][:], op=ALU.add)
                eng.tensor_tensor(ts[2][:], ts[2][:], ts[3][:], op=ALU.add)
                eng.tensor_tensor(ts[4][:], ts[4][:], ts[5][:], op=ALU.add)
                eng.tensor_tensor(ts[6][:], ts[6][:], ts[7][:], op=ALU.add)
                eng.tensor_tensor(ts[0][:], ts[0][:], ts[2][:], op=ALU.add)
                eng.tensor_tensor(ts[4][:], ts[4][:], ts[6][:], op=ALU.add)
                eng.tensor_tensor(m16[:], ts[0][:], ts[4][:], op=ALU.add)

            # aggb normalization first: evicts depend on it, and the DVE
            # list-scheduler won't hoist it past the chains below
            nc.vector.tensor_tensor(aggb[:, 0:BPC], pag0, rBt[:],
                                    op=ALU.mult)
            nc.vector.tensor_tensor(aggb[:, BPC:2 * BPC], pag1, rBt[:],
                                    op=ALU.mult)

            # All mix chains on DVE (GPSIMD cannot run TensorScalarPtr on
            # real TRN2); b0's chains are ot-split for an earlier PE start.
            mix_chain(nc.vector, 0, 0, 0, 128)
            mix_chain(nc.vector, 0, 1, 0, 128)
            mix_chain(nc.vector, 0, 0, 128, 256)
            mix_chain(nc.vector, 0, 1, 128, 256)
            mix_chain(nc.vector, 1, 0, 0, 256)
            mix_chain(nc.vector, 1, 1, 0, 256)
            mix_chain(nc.vector, 2, 0, 0, 256)
            mix_chain(nc.vector, 2, 1, 0, 256)
            mix_chain(nc.vector, 3, 0, 0, 256)
            mix_chain(nc.vector, 3, 1, 0, 256)

            # ---- x loads (SP): per sample, per it, two 2048-col halves ----
            x_t = {}
            for b in range(BPC):
                for half in range(2):
                    for it in range(N_IT):
                        t = xinp.tile([128, 2048], F16, tag=f"x{it}{half}",
                                      name=f"x_{b}_{it}_{half}")
                        x_t[(b, it, half)] = t
            for b in range(BPC):
                if b == 0:
                    # interleaved 1024-col quarters: x arrives in the order
                    # the first sample's psum chunks consume it
                    for qq in range(4):
                        for it in range(N_IT):
                            t = x_t[(b, it, qq // 2)]
                            lo = (qq % 2) * 1024
                            nc.sync.dma_start(
                                t[:, lo:lo + 1024],
                                xs[b, it * 128:(it + 1) * 128,
                                   qq * 1024:(qq + 1) * 1024])
                else:
                    for half in range(2):
                        for it in range(N_IT):
                            t = x_t[(b, it, half)]
                            nc.sync.dma_start(
                                t[:], xs[b, it * 128:(it + 1) * 128,
                                         half * 2048:(half + 1) * 2048])

            osb = {}
            for b in range(BPC):
                for ot in range(N_OT):
                    if (b, ot) == (3, 1):
                        for q in range(3):
                            osb[(b, ot, q)] = o3sbp.tile(
                                [128, 1024], F16, tag="o3",
                                name=f"o3_{b}_{ot}_{q}")
                        for sq in range(2):
                            osb[(b, ot, 3, sq)] = o3sbp.tile(
                                [128, 512], F16, tag="o3s",
                                name=f"o3s_{b}_{ot}_{sq}")
                    else:
                        for hh in range(2):
                            osb[(b, ot, hh)] = osbp.tile(
                                [128, 2048], F16, tag="o",
                                name=f"o_{b}_{ot}_{hh}")

            def emit_chunk(g, b, ot, c):
                """Matmuls + eviction for one 1024-col psum chunk."""
                ps = mps.tile([128, 1024], F32, tag=f"pm{g % 4}",
                              name=f"ps_{b}_{ot}_{c}")
                half = c // 2
                for s in range(2):
                    lo = (c % 2) * 1024 + s * 512
                    for it in range(N_IT):
                        nc.tensor.matmul(
                            ps[:, s * 512:(s + 1) * 512],
                            mh[(b, it)][:, ot * 128:(ot + 1) * 128],
                            x_t[(b, it, half)][:, lo:lo + 512],
                            start=(it == 0), stop=(it == N_IT - 1))
                bias_ap = aggb[:, ot * BPC + b:ot * BPC + b + 1]
                scale_ap = rBt[:, b:b + 1]
                if g == 31:
                    # final chunk: two 512-col halves, separate out tiles,
                    # evicted concurrently on ACT and DVE
                    nc.scalar.activation(osb[(b, ot, 3, 0)][:],
                                         ps[:, 0:512], AF.Identity,
                                         bias=bias_ap, scale=scale_ap)
                    nc.vector.tensor_scalar(osb[(b, ot, 3, 1)][:],
                                            ps[:, 512:1024],
                                            scale_ap, bias_ap,
                                            op0=ALU.mult, op1=ALU.add)
                    return
                if (b, ot) == (3, 1):
                    otile, olo = osb[(b, ot, c)], 0
                else:
                    otile, olo = osb[(b, ot, c // 2)], (c % 2) * 1024
                dst = otile[:, olo:olo + 1024]
                if ev_on_dve(g):
                    nc.vector.tensor_scalar(dst, ps[:], scale_ap, bias_ap,
                                            op0=ALU.mult, op1=ALU.add)
                else:
                    nc.scalar.activation(dst, ps[:], AF.Identity,
                                         bias=bias_ap, scale=scale_ap)

            def emit_writes(b, ot):
                # ACT: early halves (slack between its evicts); Pool: mid
                # halves deferred past its mix chains; SP: all of b3 (its x
                # stream is done by then).
                if (b, ot) == (3, 1):
                    for q in range(3):
                        orows = out[b, ot * 128:(ot + 1) * 128,
                                    q * 1024:(q + 1) * 1024]
                        nc.sync.dma_start(orows, osb[(b, ot, q)][:])
                    # final two 512-col writes on other queues so their DGE
                    # dispatch delays run concurrently with SP's last write
                    engs2 = [nc.scalar, nc.gpsimd]
                    for sq in range(2):
                        orows = out[b, ot * 128:(ot + 1) * 128,
                                    3072 + sq * 512:3072 + (sq + 1) * 512]
                        engs2[sq].dma_start(orows, osb[(b, ot, 3, sq)][:])
                    return
                for hh in range(2):
                    orows = out[b, ot * 128:(ot + 1) * 128,
                                hh * 2048:(hh + 1) * 2048]
                    src = osb[(b, ot, hh)][:]
                    nc.gpsimd.dma_start(orows, src)

            # chunk order per sample: h0 chunks of both ots first, then h1
            # (x's second half is needed 3.4us into the sample, giving the
            # serial SP x-stream slack to stay ahead of PE)
            g = 0
            B0_ORDER = ((0, 0), (1, 0), (0, 1), (1, 1),
                        (0, 2), (1, 2), (0, 3), (1, 3))
            ORDER = ((0, 0), (0, 1), (1, 0), (1, 1),
                     (0, 2), (0, 3), (1, 2), (1, 3))
            def emit_last_split():
                """Final 1024 cols as two 512-wide psum tiles on distinct
                tags so each half's eviction starts right after its own
                two matmuls instead of all four."""
                b, ot = 3, 1
                bias_ap = aggb[:, ot * BPC + b:ot * BPC + b + 1]
                scale_ap = rBt[:, b:b + 1]
                for piece, (tag, ev) in enumerate(
                        (("pm3", "act"), ("pm1", "dve"))):
                    ps = mps.tile([128, 512], F32, tag=tag,
                                  name=f"ps_last_{piece}")
                    lo = 1024 + piece * 512
                    for it in range(N_IT):
                        nc.tensor.matmul(
                            ps[:], mh[(b, it)][:, ot * 128:(ot + 1) * 128],
                            x_t[(b, it, 1)][:, lo:lo + 512],
                            start=(it == 0), stop=(it == N_IT - 1))
                    dst = osb[(b, ot, 3, piece)][:]
                    if ev == "act":
                        nc.scalar.activation(dst, ps[:], AF.Identity,
                                             bias=bias_ap, scale=scale_ap)
                    else:
                        nc.vector.tensor_scalar(dst, ps[:], scale_ap, bias_ap,
                                                op0=ALU.mult, op1=ALU.add)

            for b in range(BPC):
                for ot, c in (B0_ORDER if b == 0 else ORDER):
                    if (b, ot, c) == (3, 1, 3):
                        emit_last_split()
                    else:
                        emit_chunk(g, b, ot, c)
                    g += 1
                emit_writes(b, 0)
                emit_writes(b, 1)

    nc.compile()
    return nc


def _get_program():
    global _PROG
    if _PROG is None:
        _PROG = _build_program()
    return _PROG


def build_in_maps(inputs):
    x = np.asarray(inputs["x"], dtype=np.float32)
    emb = np.asarray(inputs["embedding"], dtype=np.float32)
    W0 = np.asarray(inputs["W0"], dtype=np.float32)
    b0 = np.asarray(inputs["b0"], dtype=np.float32)
    W1 = np.asarray(inputs["W1"], dtype=np.float32)
    b1 = np.asarray(inputs["b1"], dtype=np.float32)
    W2 = np.asarray(inputs["W2"], dtype=np.float32)
    b2 = np.asarray(inputs["b2"], dtype=np.float32)
    weight = np.asarray(inputs["weight"], dtype=np.float32)[..., 0]  # [K,O,I]
    bias_k = np.asarray(inputs["bias_k"], dtype=np.float32)

    x16 = x.astype(np.float16)

    expb2 = np.exp(b2.astype(np.float64))
    wscaled = weight * expb2[:, None, None].astype(np.float32)
    wta = np.ascontiguousarray(
        wscaled.transpose(2, 0, 1)            # [I, K, O]
        .reshape(N_IT, 128, K, OUT_C)         # [it, il, K, O]
        .transpose(1, 0, 2, 3)                # [il, it, K, O]
        .reshape(128, N_IT * K * OUT_C)).astype(np.float16)

    pa1 = np.zeros((128, P1A_COLS), dtype=np.float32)
    pa1[:, P1_W0:P1_W0 + HID] = W0
    pa1[:, P1_B0] = b0
    pa1b = np.zeros((128, P1B_COLS), dtype=np.float32)
    pa1b[:, P1B_W1:P1B_W1 + HID] = W1
    pa1b[:, P1B_B1] = b1
    pa1b[:, P1B_ONES] = 1.0

    pa2 = np.zeros((128, P2_COLS), dtype=np.float32)
    pa2[:, P2_W2:P2_W2 + K] = W2
    pa2[0:K, P2_EB2C] = expb2.astype(np.float32)
    pa2[0:K, P2_BK:P2_BK + OUT_C] = (
        bias_k * expb2[:, None]).astype(np.float32)
    pa2[0:K, P2_MSK:P2_MSK + BPC * K] = np.tile(
        np.eye(K, dtype=np.float32), (1, BPC))

    in_maps = []
    for c in range(N_CORES):
        sl = slice(c * BPC, (c + 1) * BPC)
        p1 = pa1.copy()
        p1[:, P1_EMBT:P1_EMBT + BPC] = emb[sl].T
        in_maps.append({
            "xs": np.ascontiguousarray(x16[sl]),
            "wta": wta,
            "params1a": p1,
            "params1b": pa1b,
            "params2": pa2,
        })
    return in_maps


def run(inputs, trace=False):
    nc = _get_program()
    in_maps = build_in_maps(inputs)
    res = bass_utils.run_bass_kernel_spmd(
        nc, in_maps, core_ids=list(range(N_CORES)), trace=trace)
    out = np.concatenate(
        [res.results[c]["out"] for c in range(N_CORES)], axis=0
    ).astype(np.float32)
    return out, res


def kernel(**inputs):
    out, _ = run(inputs, trace=False)
    return out
